# revision 36
# baseline (speedup 1.0000x reference)
"""Trainium2 Bass kernel for nn_Attention_2628519985914 (sparse_attention).

Math (per head h, batch b):
  mixed = w0*cos(f_q,f_k) + w1*cov(f_q,f_k)/DH + w2*var_q (x) var_k/DH
  out   = mixed @ f_v
Factorized:   out = Q_aug @ (K_aug^T @ f_v)   (one 128-dim contraction)
with Q_aug = [q_hat | q_c + a*q_var*1], K_aug = [k_hat | k_c + a*var_k*1],
a = (64/63)*sqrt(w2/(64*w1)). Centered vectors are orthogonal to the ones
vector, so the cross terms vanish and the variance rank-1 term rides inside
the covariance block: no N x N score matrix, no separate var-term matmuls.

Sharding: 8 cores = 4 batches x 2 halves of the q-token axis. K/V sides use
the full batch on each core; outputs are disjoint row-blocks -> no collectives.

Precision: all PE matmuls run in bf16 (hardware: 1 cycle/row vs 4 for fp32r
below 256 free columns; fp32 transposes are 2x). The LayerNorm inv-std is
folded into the inputs on the host (x_pre = x * rsig, in bf16), so the
PSUM->SBUF copies are plain copies. Stats (var/inv) stay fp32-ish; final
output fp32. Rel-err budget is 2e-2; bf16 lands ~1e-3.

Rotated-basis trick: per head the q/k projection weights are rotated by an
orthogonal R (Householder with last row 1/8, exact, host-side). Then
  - cosine uses g directly (rotation invariant),
  - centered coords are g[0:63] (centering == dropping the rotated 1-axis),
  - biased var = (||g||^2 - g[63]^2)/64,
  - the variance rider value c*u (u = ||g||^2 - g[63]^2) overwrites g[63].
No mean reduce, no mu, no centering pass on device.

Host precompute (cheap, O(N*D)): LayerNorm token stats folded into inputs,
the 3-way mixing weights from the weight-predictor MLP (needs a global mean
over all batches -> host avoids cross-core reduction), weight transposes.
"""

import os
import numpy as np
import ml_dtypes

BF16 = ml_dtypes.bfloat16
_DBG_RSQRT = os.environ.get("KERNEL_DEBUG_RSQRT") == "1"  # CoreSim lacks Abs_reciprocal_sqrt

B, N, DIM, H, DH = 4, 1024, 512, 8, 64
INNER = H * DH
LN_EPS = 1e-5
P = 128
NCORES = 8
TQ = N // 2          # q tokens per core
QT_TILES = TQ // P   # 4
KT_TILES = N // P    # 8

LAST_RESULT = None


def _host_ln_stats(x2d):
    x = x2d.astype(np.float64)
    mu = x.mean(-1, keepdims=True)
    var = ((x - mu) ** 2).mean(-1)
    return (1.0 / np.sqrt(var + LN_EPS)).astype(np.float32), mu[:, 0]


def _host_layernorm(x, g, b):
    x = x.astype(np.float64)
    mu = x.mean(-1, keepdims=True)
    var = ((x - mu) ** 2).mean(-1, keepdims=True)
    return (x - mu) / np.sqrt(var + LN_EPS) * g + b


def _host_mix_weights(q, k, ln_g, ln_b, W_in, wp_W1, wp_b1, wp_ln_g, wp_ln_b,
                      wp_W2, wp_b2):
    g64 = ln_g.astype(np.float64)
    b64 = ln_b.astype(np.float64)
    Wi = W_in.astype(np.float64)
    lnq = _host_layernorm(q.reshape(-1, DIM), g64, b64)
    lnk = _host_layernorm(k.reshape(-1, DIM), g64, b64)
    q_global = (lnq.mean(0) @ Wi.T).reshape(H, DH)
    k_global = (lnk.mean(0) @ Wi.T).reshape(H, DH)
    feats = np.concatenate([q_global, k_global], axis=-1)
    z = feats @ wp_W1.astype(np.float64).T + wp_b1.astype(np.float64)
    zl = _host_layernorm(z, wp_ln_g.astype(np.float64), wp_ln_b.astype(np.float64))
    h1 = np.maximum(zl, 0.0)
    logits = h1 @ wp_W2.astype(np.float64).T + wp_b2.astype(np.float64)
    e = np.exp(logits - logits.max(-1, keepdims=True))
    return (e / e.sum(-1, keepdims=True)).astype(np.float32)


# constant-pack column offsets (cst [P, 20])
C_WSC, C_ALPHA = 0, 9


def _build_nc(has_biasf, has_bout, loop_n=None):
    import concourse.bacc as bacc
    import concourse.tile as tile
    import concourse.mybir as mybir
    from concourse.masks import make_identity

    f32 = mybir.dt.float32
    bf16 = mybir.dt.bfloat16
    AX = mybir.AxisListType
    OP = mybir.AluOpType
    AF = mybir.ActivationFunctionType

    nc = bacc.Bacc()
    xq = nc.dram_tensor("xq", [DIM, TQ], bf16, kind="ExternalInput")
    xk = nc.dram_tensor("xk", [DIM, N], bf16, kind="ExternalInput")
    xv = nc.dram_tensor("xv", [DIM, N], bf16, kind="ExternalInput")
    Wc = nc.dram_tensor("Wc", [DIM, INNER], bf16, kind="ExternalInput")
    WoT = nc.dram_tensor("WoT", [INNER, DIM], bf16, kind="ExternalInput")
    cst = nc.dram_tensor("cst", [P, 20], f32, kind="ExternalInput")
    if has_biasf:
        biasf = nc.dram_tensor("biasf", [INNER], f32, kind="ExternalInput")
    if has_bout:
        bout = nc.dram_tensor("bout", [1, DIM], bf16, kind="ExternalInput")
    out = nc.dram_tensor("out", [TQ, DIM], f32, kind="ExternalOutput")

    with tile.TileContext(nc) as tc:
        with (
            tc.tile_pool(name="singles", bufs=1) as singles,
            tc.tile_pool(name="st", bufs=4) as stp,
            tc.tile_pool(name="aug", bufs=1) as augp,
            tc.tile_pool(name="pp", bufs=3, space="PSUM") as pp,
            tc.tile_pool(name="tps", bufs=2, space="PSUM") as tps,
            tc.tile_pool(name="scorep", bufs=1, space="PSUM") as scorep,
            tc.tile_pool(name="ohp", bufs=2, space="PSUM") as ohp,
        ):
            consts = {}

            def emit_consts():
                # loop-invariant: weights, constants, identity, act tables
                Wc_sb = singles.tile([P, 4, INNER], bf16, name="Wc_sb")
                Wc_r = Wc.rearrange("(c p) i -> p c i", p=P)
                nc.sync.dma_start(Wc_sb[:, 0:2, :], Wc_r[:, 0:2, :])
                nc.scalar.dma_start(Wc_sb[:, 2:4, :], Wc_r[:, 2:4, :])
                cst_sb = singles.tile([P, 20], f32, name="cst_sb")
                nc.scalar.dma_start(cst_sb[:], cst[:])
                # preload both activation tables during the initial DMA wait
                warm = singles.tile([1, 2], f32, name="warm")
                nc.vector.memset(warm[:], 1.0)
                nc.scalar.activation(warm[:, 1:2], warm[:, 0:1], func=AF.Copy)
                nc.scalar.activation(warm[:, 1:2], warm[:, 0:1],
                                     func=AF.Sqrt if _DBG_RSQRT else AF.Abs_reciprocal_sqrt)
                WoT_sb = singles.tile([P, 4, DIM], bf16, name="WoT_sb")
                nc.scalar.dma_start(WoT_sb[:], WoT.rearrange("(c p) i -> p c i", p=P))
                ident = singles.tile([P, P], bf16, name="ident")
                make_identity(nc, ident[:])
                consts.update(Wc_sb=Wc_sb, cst_sb=cst_sb, WoT_sb=WoT_sb, ident=ident)
                if has_biasf:
                    biasf_bc = singles.tile([P, INNER], f32, name="biasf_bc")
                    bc_ap = type(biasf[:])(
                        tensor=biasf[:].tensor, offset=0, ap=[[0, P], [1, INNER]]
                    )
                    nc.scalar.dma_start(biasf_bc[:], bc_ap)
                    consts["biasf_bc"] = biasf_bc
                if has_bout:
                    bout_sb = singles.tile([1, DIM], bf16, name="bout_sb")
                    nc.scalar.dma_start(bout_sb[:], bout[:])
                    ones1 = singles.tile([1, P], bf16, name="ones1")
                    nc.vector.memset(ones1[:], 1.0)
                    consts.update(bout_sb=bout_sb, ones1=ones1)

            def emit():
                Wc_sb = consts["Wc_sb"]
                cst_sb = consts["cst_sb"]
                WoT_sb = consts["WoT_sb"]
                ident = consts["ident"]
                if has_biasf:
                    biasf_bc = consts["biasf_bc"]
                if has_bout:
                    bout_sb, ones1 = consts["bout_sb"], consts["ones1"]
                wsc_sb = cst_sb[:, C_WSC:C_WSC + H]
                alpha_sb = cst_sb[:, C_ALPHA:C_ALPHA + H]

                xq_sb = singles.tile([P, 4, TQ], bf16, name="xq_sb")
                xk_sb = singles.tile([P, 4, N], bf16, name="xk_sb")
                xv_sb = singles.tile([P, 4, N], bf16, name="xv_sb")
                xq_r = xq.rearrange("(c p) t -> p c t", p=P)
                xk_r = xk.rearrange("(c p) t -> p c t", p=P)
                xv_r = xv.rearrange("(c p) t -> p c t", p=P)
                # first k/v chunks land fast, rest streams behind; spread
                # across queues (per-queue DMA bandwidth is limited on HW)
                nc.gpsimd.dma_start(xv_sb[:, :, 0:256], xv_r[:, :, 0:256])
                nc.sync.dma_start(xk_sb[:, :, 0:256], xk_r[:, :, 0:256])
                nc.gpsimd.dma_start(xv_sb[:, :, 256:512], xv_r[:, :, 256:512])
                nc.sync.dma_start(xk_sb[:, :, 256:512], xk_r[:, :, 256:512])
                nc.scalar.dma_start(xq_sb[:], xq_r[:])
                nc.gpsimd.dma_start(xv_sb[:, :, 512:], xv_r[:, :, 512:])
                nc.sync.dma_start(xk_sb[:, :, 512:], xk_r[:, :, 512:])

                def proj_psum(x_sb, t):
                    ps = pp.tile([P, INNER], f32, name="pj")
                    for dc in range(4):
                        nc.tensor.matmul(
                            ps[:], x_sb[:, dc, t * P:(t + 1) * P], Wc_sb[:, dc, :],
                            start=(dc == 0), stop=(dc == 3),
                        )
                    return ps

                # ---------------- K/V side ----------------
                # Kf/Qf[t]: [P, H, 3, DH] planes: 0 = hat (g*inv),
                # 1 = g with g[63] overwritten by the var rider, 2 = g^2.
                fv_tiles = [None] * KT_TILES
                Kf_tiles = [None] * KT_TILES
                Qf_tiles = [None] * QT_TILES
                t1_ps = scorep.tile([P, H, DH], f32, name="t1_ps")

                def v_tile(t):
                    psv = proj_psum(xv_sb, t)
                    fv_t = augp.tile([P, H, DH], bf16, name=f"fv{t}")
                    if t % 2 == 0:
                        nc.scalar.activation(fv_t[:], psv[:], func=AF.Copy)
                    else:
                        nc.vector.tensor_copy(fv_t[:], psv[:])
                    if has_biasf:
                        nc.vector.tensor_add(fv_t[:], fv_t[:],
                                             biasf_bc[:].rearrange("p (h d) -> p h d", h=H))
                    fv_tiles[t] = fv_t

                def aug_a(ps, F_t, s2_pair, parity):
                    """Part A: g copy, square, sum-of-squares reduce."""
                    g = F_t[:, :, 1, :]
                    nc.scalar.activation(g, ps[:], func=AF.Copy)
                    if has_biasf:
                        nc.vector.tensor_add(g, g,
                                             biasf_bc[:].rearrange("p (h d) -> p h d", h=H))
                    nc.gpsimd.tensor_mul(F_t[:, :, 2, :], g, g)
                    nc.vector.reduce_sum(s2_pair[:, parity, :], F_t[:, :, 2:3, :],
                                         axis=AX.X)

                def inv_pair(s2_pair, tag):
                    """One rsqrt activation covers two tiles' stats."""
                    inv2 = stp.tile([P, 2, H], f32, name="inv" + tag)
                    if _DBG_RSQRT:
                        nrm = stp.tile([P, 2, H], f32, name="nrm" + tag)
                        nc.scalar.activation(nrm[:], s2_pair[:], func=AF.Sqrt)
                        nc.vector.reciprocal(inv2[:], nrm[:])
                    else:
                        nc.scalar.activation(inv2[:], s2_pair[:],
                                             func=AF.Abs_reciprocal_sqrt)
                    return inv2

                def aug_c(F_t, s2_pair, inv2, parity, tag):
                    """Part C: hat plane + var rider into g[63]."""
                    g = F_t[:, :, 1, :]
                    nc.gpsimd.tensor_tensor(
                        F_t[:, :, 0, :], g,
                        inv2[:, parity, :, None].to_broadcast([P, H, DH]), OP.mult)
                    glast = F_t[:, :, 1, DH - 1]
                    s2 = s2_pair[:, parity, :]
                    t2 = stp.tile([P, H], f32, name="t2" + tag)
                    nc.vector.tensor_mul(t2[:], glast, glast)
                    u = stp.tile([P, H], f32, name="u" + tag)
                    nc.vector.tensor_sub(u[:], s2, t2[:])
                    nc.vector.tensor_tensor(glast, u[:], alpha_sb, OP.mult)

                def t1_t2(kt):
                    # TRN2 psum: start=True pending-zeroes the whole bank;
                    # one start for the first MM, one stop at the very end;
                    # per-byte first-touch semantics accumulate per-head
                    # regions correctly.
                    for h in range(H):
                        nc.tensor.matmul(
                            t1_ps[:, h, :], Kf_tiles[kt][:, h, 0:2, :],
                            fv_tiles[kt][:, h, :],
                            start=(kt == 0 and h == 0),
                            stop=(kt == KT_TILES - 1 and h == H - 1),
                            skip_group_check=True,
                        )

                QT_sb = [None] * H

                def transpose_head(h):
                    # Q_aug transpose for one head -> QT[h] [P(aug), TQ]
                    qt_h = singles.tile([P, TQ], bf16, name=f"QTh{h}")
                    tp = tps.tile([P, TQ], bf16, name="tp")
                    for t in range(QT_TILES):
                        nc.tensor.transpose(tp[:, t * P:(t + 1) * P],
                                            Qf_tiles[t][:, h, 0:2, :], ident[:])
                    if h % 2 == 0:
                        nc.vector.tensor_copy(qt_h[:], tp[:])
                    else:
                        nc.scalar.activation(qt_h[:], tp[:], func=AF.Copy)
                    QT_sb[h] = qt_h

                # k/v tiles paired so one rsqrt covers two tiles' stats;
                # q tiles interleave at kt 2-5; transposes spread into kt 6-7
                for kt in range(KT_TILES):
                    par = kt % 2
                    if par == 0:
                        s2k = stp.tile([P, 2, H], f32, name=f"s2k{kt // 2}")
                    v_tile(kt)
                    psk = proj_psum(xk_sb, kt)
                    Kf_tiles[kt] = augp.tile([P, H, 3, DH], bf16, name=f"Kf{kt}")
                    aug_a(psk, Kf_tiles[kt], s2k, par)
                    if par == 1:
                        invk = inv_pair(s2k, f"k{kt // 2}")
                        aug_c(Kf_tiles[kt - 1], s2k, invk, 0, f"k{kt - 1}")
                        aug_c(Kf_tiles[kt], s2k, invk, 1, f"k{kt}")
                        t1_t2(kt - 1)
                        t1_t2(kt)
                    if 2 <= kt <= 5:
                        qt = kt - 2
                        qpar = qt % 2
                        if qpar == 0:
                            s2q = stp.tile([P, 2, H], f32, name=f"s2q{qt // 2}")
                        psq = proj_psum(xq_sb, qt)
                        Qf_tiles[qt] = augp.tile([P, H, 3, DH], bf16, name=f"Qf{qt}")
                        aug_a(psq, Qf_tiles[qt], s2q, qpar)
                        if qpar == 1:
                            invq = inv_pair(s2q, f"q{qt // 2}")
                            aug_c(Qf_tiles[qt - 1], s2q, invq, 0, f"q{qt - 1}")
                            aug_c(Qf_tiles[qt], s2q, invq, 1, f"q{qt}")
                    if kt == 6:
                        for h in range(4):
                            transpose_head(h)
                for h in range(4, H):
                    transpose_head(h)

                # ---------------- scores (factorized) ----------------
                T1S = singles.tile([P, H, DH], bf16, name="T1S")
                for half in range(2):
                    hs = slice(half * 4, (half + 1) * 4)
                    nc.vector.tensor_tensor(
                        T1S[:, hs, :], t1_ps[:, hs, :],
                        wsc_sb[:, hs, None].to_broadcast([P, 4, DH]), OP.mult)
                # ---------------- out heads + final projection ----------------
                AT = []
                for j in range(4):
                    at_j = singles.tile([P, TQ], bf16, name=f"AT{j}")
                    AT.append(at_j)
                for h in range(H):
                    j, s = divmod(h, 2)
                    if h % 3 == 2:
                        oh_ps = scorep.tile([DH, TQ], f32, name="t1_ps")
                    else:
                        oh_ps = ohp.tile([DH, TQ], f32, name="oh")
                    nc.tensor.matmul(oh_ps[:], T1S[:, h, :], QT_sb[h][:],
                                     start=True, stop=True, skip_group_check=True)
                    if h % 2 == 0:
                        nc.scalar.activation(AT[j][s * DH:(s + 1) * DH, :], oh_ps[:],
                                             func=AF.Copy)
                    else:
                        nc.vector.tensor_copy(AT[j][s * DH:(s + 1) * DH, :], oh_ps[:])

                # t-major: each token-block's accumulation finishes early so
                # its copy + store DMA overlap the remaining matmuls
                out_r = out.rearrange("(c p) d -> p c d", p=P)
                o_all = singles.tile([P, QT_TILES, DIM], f32, name="o_all")
                for t in range(QT_TILES):
                    fps = pp.tile([P, DIM], f32, name="pj")
                    for j in range(4):
                        nc.tensor.matmul(fps[:], AT[j][:, t * P:(t + 1) * P],
                                         WoT_sb[:, j, :],
                                         start=(j == 0),
                                         stop=(j == 3 and not has_bout),
                                         skip_group_check=True)
                    if has_bout:
                        nc.tensor.matmul(fps[:], ones1[:], bout_sb[:],
                                         start=False, stop=True,
                                         skip_group_check=True)
                    if t % 2 == 0:
                        nc.vector.tensor_copy(o_all[:, t, :], fps[:])
                    else:
                        nc.scalar.activation(o_all[:, t, :], fps[:], func=AF.Copy)
                    eng = nc.sync if t % 2 == 0 else nc.scalar
                    eng.dma_start(out_r[:, t, :], o_all[:, t, :])
                if os.environ.get("KERNEL_DEBUG_TAPS") == "1":
                    taps = {
                        "d_fv0": fv_tiles[0][:], "d_Kf0": Kf_tiles[0][:],
                        "d_Qf0": Qf_tiles[0][:], "d_T1S": T1S[:],
                        "d_QT0": QT_sb[0][:], "d_AT0": AT[0][:],
                    }
                    for nm, ap in taps.items():
                        dt_ = nc.dram_tensor(nm, list(ap.shape), ap.dtype,
                                             kind="ExternalOutput")
                        nc.sync.dma_start(dt_[:], ap)

            emit_consts()
            if loop_n is None:
                emit()
            else:
                import concourse.mybir as _mb
                with tc.For_i(0, loop_n, 1, staggered_reset=True, hint_engines=(
                        _mb.EngineType.PE, _mb.EngineType.DVE,
                        _mb.EngineType.Activation, _mb.EngineType.SP,
                        _mb.EngineType.Pool)):
                    emit()

    nc.compile()
    return nc


_NC_CACHE = {}


def _prepare(q, k, v, ln_g, ln_b, W_in, W_out, b_out,
             wp_W1, wp_b1, wp_ln_g, wp_ln_b, wp_W2, wp_b2):
    q = np.asarray(q, np.float32)
    k = np.asarray(k, np.float32)
    v = np.asarray(v, np.float32)
    ln_g = np.asarray(ln_g, np.float32)
    ln_b = np.asarray(ln_b, np.float32)
    W_in = np.asarray(W_in, np.float32)
    W_out = np.asarray(W_out, np.float32)
    b_out = np.asarray(b_out, np.float32)

    w = _host_mix_weights(q, k, ln_g, ln_b, W_in,
                          np.asarray(wp_W1, np.float32), np.asarray(wp_b1, np.float32),
                          np.asarray(wp_ln_g, np.float32), np.asarray(wp_ln_b, np.float32),
                          np.asarray(wp_W2, np.float32), np.asarray(wp_b2, np.float32))

    W_eff = (ln_g[:, None].astype(np.float64) * W_in.astype(np.float64).T)
    wsum = W_eff.sum(0)
    W_c64 = W_eff - wsum[None, :] / DIM
    bias_f64 = ln_b.astype(np.float64) @ W_in.astype(np.float64).T

    # orthogonal per-head rotation R (Householder, symmetric): last row 1/8
    e = np.zeros(DH); e[DH - 1] = 1.0
    wv = np.ones(DH) / np.sqrt(DH)
    uh = e - wv
    R = np.eye(DH) - 2.0 * np.outer(uh, uh) / (uh @ uh)
    W_rot = W_c64.reshape(DIM, H, DH) @ R.T          # g = R @ f per head
    W_rot = W_rot.reshape(DIM, INNER)
    W_c = W_rot.astype(BF16)
    bias_f = (bias_f64.reshape(H, DH) @ R.T).reshape(INNER).astype(np.float32)
    has_biasf = bool(np.any(bias_f != 0))
    has_bout = bool(np.any(b_out != 0))
    # v is projected with the rotated weights too; un-rotate via W_out:
    # out_h = (mixed @ fv_rot) @ (R @ W_out_h^T)  (R symmetric orthogonal)
    WoT64 = W_out.T.astype(np.float64).reshape(H, DH, DIM)
    WoT64 = np.einsum('de,hec->hdc', R, WoT64)
    W_outT = np.ascontiguousarray(WoT64.reshape(INNER, DIM)).astype(BF16)

    rsig_q, _ = _host_ln_stats(q.reshape(-1, DIM))
    rsig_k, _ = _host_ln_stats(k.reshape(-1, DIM))
    rsig_v, _ = _host_ln_stats(v.reshape(-1, DIM))
    rsig_q = rsig_q.reshape(B, N)
    rsig_k = rsig_k.reshape(B, N)
    rsig_v = rsig_v.reshape(B, N)

    key = (has_biasf, has_bout)
    if key not in _NC_CACHE:
        _NC_CACHE[key] = _build_nc(has_biasf, has_bout)
    nc = _NC_CACHE[key]

    in_maps = []
    for c in range(NCORES):
        b, half = divmod(c, 2)
        tsl = slice(half * TQ, (half + 1) * TQ)
        cstm = np.zeros((P, 20), np.float32)
        cstm[:DH, C_WSC:C_WSC + H] = w[:, 0][None, :]
        cstm[DH:, C_WSC:C_WSC + H] = (w[:, 1] / DH)[None, :]
        w64 = w.astype(np.float64)
        # rider coeff: (w1/DH)*(c*u_q)*(c*u_k) == (w2/DH)*var_u_q*var_u_k
        # with u = DH*var_biased and var_unbiased = var_biased*DH/(DH-1)
        alpha = np.sqrt(w64[:, 2] / np.maximum(w64[:, 1], 1e-30)) / (DH - 1)
        cstm[:, C_ALPHA:C_ALPHA + H] = alpha.astype(np.float32)[None, :]
        m = {
            "xq": (q[b, tsl, :].T * rsig_q[b, tsl][None, :]).astype(BF16),
            "xk": (k[b].T * rsig_k[b][None, :]).astype(BF16),
            "xv": (v[b].T * rsig_v[b][None, :]).astype(BF16),
            "Wc": W_c,
            "WoT": W_outT,
            "cst": cstm,
        }
        if has_biasf:
            m["biasf"] = bias_f
        if has_bout:
            m["bout"] = b_out[None, :].astype(BF16)
        in_maps.append(m)

    return nc, in_maps


def _assemble(results):
    full = np.empty((B, N, DIM), np.float32)
    for c in range(NCORES):
        b, half = divmod(c, 2)
        full[b, half * TQ:(half + 1) * TQ, :] = results[c]["out"]
    return full


def kernel(q, k, v, ln_g, ln_b, W_in, W_out, b_out,
           wp_W1, wp_b1, wp_ln_g, wp_ln_b, wp_W2, wp_b2):
    global LAST_RESULT
    from concourse.bass_utils import run_bass_kernel_spmd

    nc, in_maps = _prepare(q, k, v, ln_g, ln_b, W_in, W_out, b_out,
                           wp_W1, wp_b1, wp_ln_g, wp_ln_b, wp_W2, wp_b2)
    res = run_bass_kernel_spmd(nc, in_maps, core_ids=list(range(NCORES)))
    LAST_RESULT = res
    return _assemble(res.results)
